# revision 16
# baseline (speedup 1.0000x reference)
"""CPC InfoNCE loss kernel for 8x Trainium2 NeuronCores.

Math (reference):
    x_pred = y @ W.T + b                       [N, D]
    xpn    = x_pred / ||x_pred||_rows          [N, D]
    xn     = x / ||x||_rows                    [N, D]
    pos_i  = xn_i . xpn_i
    neg_i  = logsumexp_j(xn_i . xpn_j)
    loss   = -mean(pos - neg)

Strategy (data-parallel over N across 8 cores, two SPMD dispatches):

  Dispatch 1 (bf16): core i computes its row-shard of x_pred.  The bias is
    folded into the matmul by augmenting the contraction dim on the host:
    y' = [y | 1 | 0...], W' = [W | b | 0...]  (K: 1024 -> 1152), so the PSUM
    result needs no eviction pass — the ACT engine squares it directly for
    row norms, scales it to normalized bf16 output, and the DVE computes
    pos via an elementwise product + row reduction.  rx = 1/||x_row|| is
    also produced here.

  Host: gather the 8 normalized shards, transpose to [D, N], scale by 32
    and quantize to fp8e4m3 (cosine-similarity scores tolerate fp8; 32x
    keeps unit-norm entries in e4m3's normal range; the 1/32 is folded into
    the per-row exp scale).

  Dispatch 2 (fp8 + DoubleRow): core i computes its scores block
    u = x8_shard @ xpn8^T with DoubleRow matmuls (2 fp8 contraction rows
    per PE cell -> half the matmul instructions), then exp(u * rx_i/32)
    fused on the ACT engine (per-partition scale + row-accumulate), one Ln
    at the end -> neg rows.  exp without max-subtraction is safe: scores
    are cosine similarities in [-1, 1].

  Host: loss = mean(neg) - mean(pos).

All large tensors are pre-swizzled on the host into partition-major
[128, *] layouts so each tensor (or pipeline chunk) loads in one large
DMA (~2us fixed cost per DMA otherwise dominates), split across the two
HWDGE rings (sync + scalar) and the SWDGE ring (gpsimd).
"""

import sys

if "/opt/trn_rl_repo" not in sys.path:
    sys.path.insert(0, "/opt/trn_rl_repo")

import numpy as np
import ml_dtypes

import concourse.bass as bass
import concourse.bacc as bacc
import concourse.mybir as mybir
import concourse.tile as tile
from concourse.bass_utils import run_bass_kernel_spmd

BF16 = mybir.dt.bfloat16
F32 = mybir.dt.float32
F8 = mybir.dt.float8e4
NP_BF16 = ml_dtypes.bfloat16
NP_F8 = ml_dtypes.float8_e4m3fn

N_CORES = 8
N = 8192
D = 1024
NS = N // N_CORES  # rows per core = 1024
P = 128  # partitions
NB = NS // P  # row blocks per core = 8
DT = D // P  # contraction tiles = 8
DTA = DT + 1  # augmented contraction tiles (bias row + zero pad)
NTP = DT // 2  # DoubleRow tile pairs = 4
MM_N = 512  # moving free dim per matmul (one fp32 PSUM bank)
JC_W = 2048  # scores column chunk (4 PSUM banks, one ACT call)
N_JC = N // JC_W  # 4 chunks of the full N columns
XPN_SCALE = 32.0  # fp8 pre-scale for unit-norm rows


def _swizzle_pm(a):
    """[R*128, C] row-major -> [128, R*C] partition-major (tile r at columns
    r*C:(r+1)*C), so the whole tensor loads as one [128, R*C] DMA."""
    r8, c = a.shape[0] // P, a.shape[1]
    return np.ascontiguousarray(
        a.reshape(r8, P, c).transpose(1, 0, 2).reshape(P, r8 * c))


def _unswizzle_pm(a, r8):
    """Inverse of _swizzle_pm."""
    c = a.shape[1] // r8
    return np.ascontiguousarray(
        a.reshape(P, r8, c).transpose(1, 0, 2).reshape(r8 * P, c))


def _build_dispatch1():
    nc = bacc.Bacc("TRN2", target_bir_lowering=False, debug=False,
                   num_devices=N_CORES)
    yT_d = nc.dram_tensor("yT", [P, DTA * NS], BF16, kind="ExternalInput")
    wT_d = nc.dram_tensor("wT", [P, DTA * D], BF16, kind="ExternalInput")
    x_d = nc.dram_tensor("xin", [P, NB * D], BF16, kind="ExternalInput")
    xpn_d = nc.dram_tensor("xpn", [P, NB * D], BF16, kind="ExternalOutput")
    # stat: columns [0:NB] = pos, [NB:2NB] = rx
    stat_d = nc.dram_tensor("stat", [P, 2 * NB], F32, kind="ExternalOutput")

    with tile.TileContext(nc) as tc:
        with (
            tc.tile_pool(name="persist", bufs=1) as persist,
            tc.tile_pool(name="scratch", bufs=3) as scratch,
            tc.tile_pool(name="stats", bufs=NB) as stats,
            tc.tile_pool(name="psum", bufs=3,
                         space=bass.MemorySpace.PSUM) as psum,
        ):
            # split loads across both HWDGE rings for parallel transfer
            yts, wts = [], []
            for t in range(DTA):
                yt = persist.tile([P, NS], BF16, tag=f"yT{t}")
                nc.sync.dma_start(out=yt[:], in_=yT_d[:, t * NS:(t + 1) * NS])
                yts.append(yt)
                wt = persist.tile([P, D], BF16, tag=f"wT{t}")
                nc.scalar.dma_start(out=wt[:], in_=wT_d[:, t * D:(t + 1) * D])
                wts.append(wt)
            x_sb = persist.tile([P, NB * D], BF16, tag="x")
            nc.gpsimd.dma_start(out=x_sb[:], in_=x_d[:])

            xpn_all = persist.tile([P, NB * D], BF16, tag="xpn_all")
            stat_all = persist.tile([P, 2 * NB], F32, tag="stat_all")

            for nb in range(NB):
                pp = psum.tile([P, D], F32, tag="pp")
                for t in range(DTA):
                    lhsT = yts[t][:, nb * P:(nb + 1) * P]
                    for c in range(D // MM_N):
                        nc.tensor.matmul(
                            pp[:, c * MM_N:(c + 1) * MM_N], lhsT,
                            wts[t][:, c * MM_N:(c + 1) * MM_N],
                            start=(t == 0), stop=(t == DTA - 1))

                # row sumsq -> 1/norm (ACT reads PSUM directly)
                sq = scratch.tile([P, D], F32, tag="sq")
                ss = stats.tile([P, 1], F32, tag="ss")
                nc.scalar.activation(sq[:], pp[:],
                                     mybir.ActivationFunctionType.Square,
                                     accum_out=ss[:])
                nrm = stats.tile([P, 1], F32, tag="nrm")
                nc.scalar.activation(nrm[:], ss[:],
                                     mybir.ActivationFunctionType.Sqrt)
                rpn = stats.tile([P, 1], F32, tag="rpn")
                nc.vector.reciprocal(rpn[:], nrm[:])

                # normalized rows -> bf16, streamed out per block
                nc.scalar.mul(xpn_all[:, nb * D:(nb + 1) * D], pp[:], rpn[:])
                nc.sync.dma_start(out=xpn_d[:, nb * D:(nb + 1) * D],
                                  in_=xpn_all[:, nb * D:(nb + 1) * D])

                # rx = 1/||x_row||
                xsq = scratch.tile([P, D], F32, tag="sq")
                ssx = stats.tile([P, 1], F32, tag="ssx")
                nc.scalar.activation(xsq[:], x_sb[:, nb * D:(nb + 1) * D],
                                     mybir.ActivationFunctionType.Square,
                                     accum_out=ssx[:])
                nx = stats.tile([P, 1], F32, tag="nx")
                nc.scalar.activation(nx[:], ssx[:],
                                     mybir.ActivationFunctionType.Sqrt)
                rx = stats.tile([P, 1], F32, tag="rx")
                nc.vector.reciprocal(rx[:], nx[:])
                nc.vector.tensor_copy(stat_all[:, NB + nb:NB + nb + 1], rx[:])

                # pos = (x_row . x_pred_row) * rpn * rx
                pd_scr = scratch.tile([P, D], F32, tag="sq")
                nc.vector.tensor_mul(pd_scr[:], x_sb[:, nb * D:(nb + 1) * D],
                                     pp[:])
                posdot = stats.tile([P, 1], F32, tag="posdot")
                nc.vector.reduce_sum(posdot[:], pd_scr[:],
                                     axis=mybir.AxisListType.X)
                t1 = stats.tile([P, 1], F32, tag="t1")
                nc.vector.tensor_mul(t1[:], posdot[:], rpn[:])
                nc.vector.tensor_mul(stat_all[:, nb:nb + 1], t1[:], rx[:])

            nc.gpsimd.dma_start(out=stat_d[:], in_=stat_all[:])

    nc.compile()
    return nc


def _build_dispatch2():
    nc = bacc.Bacc("TRN2", target_bir_lowering=False, debug=False,
                   num_devices=N_CORES)
    xT_d = nc.dram_tensor("xT", [P, DT * NS], F8, kind="ExternalInput")
    # layout: [p][jc][tp][o][c] blocks, each (jc, tp) block = [128, 2*JC_W]
    xpnT_d = nc.dram_tensor("xpnT", [P, DT * N], F8, kind="ExternalInput")
    rx_d = nc.dram_tensor("rxv", [P, NB], F32, kind="ExternalInput")
    neg_d = nc.dram_tensor("negv", [P, NB], F32, kind="ExternalOutput")

    with tile.TileContext(nc) as tc:
        with (
            tc.tile_pool(name="persist", bufs=1) as persist,
            tc.tile_pool(name="esc", bufs=2) as escp,
            tc.tile_pool(name="psum", bufs=2,
                         space=bass.MemorySpace.PSUM) as psum,
        ):
            rx_sb = persist.tile([P, NB], F32, tag="rx")
            nc.gpsimd.dma_start(out=rx_sb[:], in_=rx_d[:])
            xT_sb = persist.tile([P, DT * NS], F8, tag="xT")
            nc.gpsimd.dma_start(out=xT_sb[:], in_=xT_d[:])
            xT3 = xT_sb[:].rearrange("p (t m) -> p t m", t=DT)

            separts = persist.tile([P, NB * N_JC], F32, tag="separts")

            # jc-major: compute on chunk jc overlaps the DMA of chunk jc+1
            for jc in range(N_JC):
                xp_tp = []
                for tp in range(NTP):
                    base = (jc * NTP + tp) * 2 * JC_W
                    xp = persist.tile([P, 2 * JC_W], F8, tag=f"xpnT{jc}_{tp}")
                    nc.sync.dma_start(out=xp[:],
                                      in_=xpnT_d[:, base:base + 2 * JC_W])
                    xp_tp.append(xp)
                for ib in range(NB):
                    ps = psum.tile([P, JC_W], F32, tag="ps")
                    for tp in range(NTP):
                        lhs3 = xT3[:, 2 * tp:2 * tp + 2, ib * P:(ib + 1) * P]
                        rhs3 = xp_tp[tp][:].rearrange("p (o c) -> p o c", o=2)
                        for c in range(JC_W // MM_N):
                            nc.tensor.matmul(
                                ps[:, c * MM_N:(c + 1) * MM_N],
                                lhs3,
                                rhs3[:, :, c * MM_N:(c + 1) * MM_N],
                                start=(tp == 0), stop=(tp == NTP - 1),
                                perf_mode=mybir.MatmulPerfMode.DoubleRow)
                    esc = escp.tile([P, JC_W], BF16, tag="esc")
                    nc.scalar.activation(
                        esc[:], ps[:], mybir.ActivationFunctionType.Exp,
                        scale=rx_sb[:, ib:ib + 1],
                        accum_out=separts[:, ib * N_JC + jc:
                                          ib * N_JC + jc + 1])

            # one reduction + one Ln for all row blocks (single table load)
            se_all = persist.tile([P, NB], F32, tag="se_all")
            nc.vector.reduce_sum(
                se_all[:], separts[:].rearrange("p (i j) -> p i j", j=N_JC),
                axis=mybir.AxisListType.X)
            neg_sb = persist.tile([P, NB], F32, tag="neg_sb")
            nc.scalar.activation(neg_sb[:], se_all[:],
                                 mybir.ActivationFunctionType.Ln)
            nc.sync.dma_start(out=neg_d[:], in_=neg_sb[:])

    nc.compile()
    return nc


_NC1 = None
_NC2 = None


def _programs():
    global _NC1, _NC2
    if _NC1 is None:
        _NC1 = _build_dispatch1()
    if _NC2 is None:
        _NC2 = _build_dispatch2()
    return _NC1, _NC2


def kernel(x, y, W, b, _timing=None):
    assert x.shape == (N, D) and y.shape == (N, D)
    assert W.shape == (D, D) and b.shape == (D,)
    nc1, nc2 = _programs()
    core_ids = list(range(N_CORES))

    x = np.asarray(x, dtype=np.float32)
    y_bf = np.asarray(y, dtype=np.float32).astype(NP_BF16)
    x_bf = x.astype(NP_BF16)
    x_f8 = x.astype(NP_F8)

    # augmented W' = [W | b | zeros] transposed: [DTA*128, D]
    wTa = np.zeros((DTA * P, D), dtype=NP_BF16)
    wTa[:D] = np.asarray(W, dtype=np.float32).astype(NP_BF16).T
    wTa[D] = np.asarray(b, dtype=np.float32).astype(NP_BF16)
    wTa_sw = _swizzle_pm(wTa)

    in_maps1 = []
    for i in range(N_CORES):
        sl = slice(i * NS, (i + 1) * NS)
        yTa = np.zeros((DTA * P, NS), dtype=NP_BF16)
        yTa[:D] = y_bf[sl].T
        yTa[D] = NP_BF16(1.0)
        in_maps1.append({
            "yT": _swizzle_pm(yTa),
            "wT": wTa_sw,
            "xin": _swizzle_pm(x_bf[sl]),
        })
    r1 = run_bass_kernel_spmd(nc1, in_maps1, core_ids)
    if _timing is not None:
        _timing["d1"] = r1.exec_time_ns

    xpn = np.concatenate(
        [_unswizzle_pm(r1.results[i]["xpn"].astype(NP_BF16, copy=False), NB)
         for i in range(N_CORES)], axis=0)          # [N, D] bf16
    pos = np.concatenate(
        [r1.results[i]["stat"][:, :NB].T.ravel() for i in range(N_CORES)])

    # fp8 scores operand: 32 * xpn^T, swizzled to [p][jc][tp][o][c]
    xpn8T = np.ascontiguousarray(
        (xpn.astype(np.float32) * XPN_SCALE).astype(NP_F8).T)   # [D, N]
    xpnT_sw = np.ascontiguousarray(
        xpn8T.reshape(NTP, 2, P, N_JC, JC_W).transpose(2, 3, 0, 1, 4)
        .reshape(P, DT * N))

    in_maps2 = []
    for i in range(N_CORES):
        sl = slice(i * NS, (i + 1) * NS)
        rx_sw = np.ascontiguousarray(
            r1.results[i]["stat"][:, NB:] / np.float32(XPN_SCALE))
        in_maps2.append({
            "xT": _swizzle_pm(np.ascontiguousarray(x_f8[sl].T)),
            "xpnT": xpnT_sw,
            "rxv": rx_sw,
        })
    r2 = run_bass_kernel_spmd(nc2, in_maps2, core_ids)
    if _timing is not None:
        _timing["d2"] = r2.exec_time_ns

    neg = np.concatenate(
        [r2.results[i]["negv"].T.ravel() for i in range(N_CORES)])
    loss = np.mean(neg.astype(np.float64)) - np.mean(pos.astype(np.float64))
    return np.asarray(loss, dtype=np.float32)


# revision 19
# speedup vs baseline: 1.0128x; 1.0128x over previous
"""CPC InfoNCE loss kernel for 8x Trainium2 NeuronCores.

Math (reference):
    x_pred = y @ W.T + b                       [N, D]
    xpn    = x_pred / ||x_pred||_rows          [N, D]
    xn     = x / ||x||_rows                    [N, D]
    pos_i  = xn_i . xpn_i
    neg_i  = logsumexp_j(xn_i . xpn_j)
    loss   = -mean(pos - neg)

Strategy (data-parallel over N across 8 cores, two SPMD dispatches):

  Dispatch 1 (bf16): core i computes its row-shard of x_pred.  The bias is
    folded into the matmul by augmenting the contraction dim on the host:
    y' = [y | 1 | 0...], W' = [W | b | 0...]  (K: 1024 -> 1152), so the PSUM
    result needs no eviction pass — the ACT engine squares it directly for
    row norms, scales it to normalized bf16 output, and the DVE computes
    pos via an elementwise product + row reduction.  rx = 1/||x_row|| is
    also produced here.

  Host: gather the 8 normalized shards, transpose to [D, N], scale by 32
    and quantize to fp8e4m3 (cosine-similarity scores tolerate fp8; 32x
    keeps unit-norm entries in e4m3's normal range; the 1/32 is folded into
    the per-row exp scale).

  Dispatch 2 (fp8 + DoubleRow): core i computes its scores block
    u = x8_shard @ xpn8^T with DoubleRow matmuls (2 fp8 contraction rows
    per PE cell -> half the matmul instructions), then exp(u * rx_i/32)
    fused on the ACT engine (per-partition scale + row-accumulate), one Ln
    at the end -> neg rows.  exp without max-subtraction is safe: scores
    are cosine similarities in [-1, 1].

  Host: loss = mean(neg) - mean(pos).

All large tensors are pre-swizzled on the host into partition-major
[128, *] layouts so each tensor (or pipeline chunk) loads in one large
DMA (~2us fixed cost per DMA otherwise dominates), split across the two
HWDGE rings (sync + scalar) and the SWDGE ring (gpsimd).
"""

import sys

if "/opt/trn_rl_repo" not in sys.path:
    sys.path.insert(0, "/opt/trn_rl_repo")

import numpy as np
import ml_dtypes

import concourse.bass as bass
import concourse.bacc as bacc
import concourse.mybir as mybir
import concourse.tile as tile
from concourse.bass_utils import run_bass_kernel_spmd

BF16 = mybir.dt.bfloat16
F32 = mybir.dt.float32
F8 = mybir.dt.float8e4
NP_BF16 = ml_dtypes.bfloat16
NP_F8 = ml_dtypes.float8_e4m3fn

N_CORES = 8
N = 8192
D = 1024
NS = N // N_CORES  # rows per core = 1024
P = 128  # partitions
NB = NS // P  # row blocks per core = 8
DT = D // P  # contraction tiles = 8
DTA = DT + 1  # augmented contraction tiles (bias row + zero pad)
NTP = DT // 2  # DoubleRow tile pairs = 4
MM_N = 512  # moving free dim per matmul (one fp32 PSUM bank)
JC_W = 2048  # scores column chunk (4 PSUM banks, one ACT call)
N_JC = N // JC_W  # 4 chunks of the full N columns
XPN_SCALE = 32.0  # fp8 pre-scale for unit-norm rows


def _swizzle_pm(a):
    """[R*128, C] row-major -> [128, R*C] partition-major (tile r at columns
    r*C:(r+1)*C), so the whole tensor loads as one [128, R*C] DMA."""
    r8, c = a.shape[0] // P, a.shape[1]
    return np.ascontiguousarray(
        a.reshape(r8, P, c).transpose(1, 0, 2).reshape(P, r8 * c))


def _unswizzle_pm(a, r8):
    """Inverse of _swizzle_pm."""
    c = a.shape[1] // r8
    return np.ascontiguousarray(
        a.reshape(P, r8, c).transpose(1, 0, 2).reshape(r8 * P, c))


def _build_dispatch1():
    nc = bacc.Bacc("TRN2", target_bir_lowering=False, debug=False,
                   num_devices=N_CORES)
    yT_d = nc.dram_tensor("yT", [P, DTA * NS], BF16, kind="ExternalInput")
    wT_d = nc.dram_tensor("wT", [P, DTA * D], BF16, kind="ExternalInput")
    x_d = nc.dram_tensor("xin", [P, NB * D], BF16, kind="ExternalInput")
    xpn_d = nc.dram_tensor("xpn", [P, NB * D], BF16, kind="ExternalOutput")
    # stat: columns [0:NB] = pos, [NB:2NB] = rx
    stat_d = nc.dram_tensor("stat", [P, 2 * NB], F32, kind="ExternalOutput")

    with tile.TileContext(nc) as tc:
        with (
            tc.tile_pool(name="persist", bufs=1) as persist,
            tc.tile_pool(name="scratch", bufs=3) as scratch,
            tc.tile_pool(name="stats", bufs=NB) as stats,
            tc.tile_pool(name="psum", bufs=3,
                         space=bass.MemorySpace.PSUM) as psum,
        ):
            # split loads across both HWDGE rings for parallel transfer
            yts, wts = [], []
            for t in range(DTA):
                yt = persist.tile([P, NS], BF16, tag=f"yT{t}")
                nc.sync.dma_start(out=yt[:], in_=yT_d[:, t * NS:(t + 1) * NS])
                yts.append(yt)
                wt = persist.tile([P, D], BF16, tag=f"wT{t}")
                nc.scalar.dma_start(out=wt[:], in_=wT_d[:, t * D:(t + 1) * D])
                wts.append(wt)
            x_sb = persist.tile([P, NB * D], BF16, tag="x")
            nc.gpsimd.dma_start(out=x_sb[:], in_=x_d[:])

            xpn_all = persist.tile([P, NB * D], BF16, tag="xpn_all")
            stat_all = persist.tile([P, 2 * NB], F32, tag="stat_all")

            for nb in range(NB):
                pp = psum.tile([P, D], F32, tag="pp")
                for t in range(DTA):
                    lhsT = yts[t][:, nb * P:(nb + 1) * P]
                    for c in range(D // MM_N):
                        nc.tensor.matmul(
                            pp[:, c * MM_N:(c + 1) * MM_N], lhsT,
                            wts[t][:, c * MM_N:(c + 1) * MM_N],
                            start=(t == 0), stop=(t == DTA - 1))

                # row sumsq -> 1/norm (ACT reads PSUM directly)
                sq = scratch.tile([P, D], F32, tag="sq")
                ss = stats.tile([P, 1], F32, tag="ss")
                nc.scalar.activation(sq[:], pp[:],
                                     mybir.ActivationFunctionType.Square,
                                     accum_out=ss[:])
                nrm = stats.tile([P, 1], F32, tag="nrm")
                nc.scalar.activation(nrm[:], ss[:],
                                     mybir.ActivationFunctionType.Sqrt)
                rpn = stats.tile([P, 1], F32, tag="rpn")
                nc.vector.reciprocal(rpn[:], nrm[:])

                # normalized rows -> bf16, streamed out per block
                nc.scalar.mul(xpn_all[:, nb * D:(nb + 1) * D], pp[:], rpn[:])
                nc.sync.dma_start(out=xpn_d[:, nb * D:(nb + 1) * D],
                                  in_=xpn_all[:, nb * D:(nb + 1) * D])

                # rx = 1/||x_row||
                xsq = scratch.tile([P, D], F32, tag="sq")
                ssx = stats.tile([P, 1], F32, tag="ssx")
                nc.scalar.activation(xsq[:], x_sb[:, nb * D:(nb + 1) * D],
                                     mybir.ActivationFunctionType.Square,
                                     accum_out=ssx[:])
                nx = stats.tile([P, 1], F32, tag="nx")
                nc.scalar.activation(nx[:], ssx[:],
                                     mybir.ActivationFunctionType.Sqrt)
                rx = stats.tile([P, 1], F32, tag="rx")
                nc.vector.reciprocal(rx[:], nx[:])
                nc.vector.tensor_copy(stat_all[:, NB + nb:NB + nb + 1], rx[:])

                # pos = (x_row . x_pred_row) * rpn * rx
                pd_scr = scratch.tile([P, D], F32, tag="sq")
                nc.vector.tensor_mul(pd_scr[:], x_sb[:, nb * D:(nb + 1) * D],
                                     pp[:])
                posdot = stats.tile([P, 1], F32, tag="posdot")
                nc.vector.reduce_sum(posdot[:], pd_scr[:],
                                     axis=mybir.AxisListType.X)
                t1 = stats.tile([P, 1], F32, tag="t1")
                nc.vector.tensor_mul(t1[:], posdot[:], rpn[:])
                nc.vector.tensor_mul(stat_all[:, nb:nb + 1], t1[:], rx[:])

            nc.gpsimd.dma_start(out=stat_d[:], in_=stat_all[:])

    nc.compile()
    return nc


def _build_dispatch2():
    nc = bacc.Bacc("TRN2", target_bir_lowering=False, debug=False,
                   num_devices=N_CORES)
    xT_d = nc.dram_tensor("xT", [P, DT * NS], F8, kind="ExternalInput")
    # layout: [p][jc][tp][o][c] blocks, each (jc, tp) block = [128, 2*JC_W]
    xpnT_d = nc.dram_tensor("xpnT", [P, DT * N], F8, kind="ExternalInput")
    rx_d = nc.dram_tensor("rxv", [P, NB], F32, kind="ExternalInput")
    neg_d = nc.dram_tensor("negv", [P, NB], F32, kind="ExternalOutput")

    with tile.TileContext(nc) as tc:
        with (
            tc.tile_pool(name="persist", bufs=1) as persist,
            tc.tile_pool(name="esc", bufs=2) as escp,
            tc.tile_pool(name="psum", bufs=2,
                         space=bass.MemorySpace.PSUM) as psum,
        ):
            rx_sb = persist.tile([P, NB], F32, tag="rx")
            nc.gpsimd.dma_start(out=rx_sb[:], in_=rx_d[:])
            # x^T loaded as per-ib chunks (ib-major host layout) so the first
            # row block's matmuls only wait on a 128 KB load
            xib = []
            for ib in range(NB):
                xt = persist.tile([P, DT * P], F8, tag=f"xib{ib}",
                                  name=f"xib{ib}")
                nc.gpsimd.dma_start(
                    out=xt[:], in_=xT_d[:, ib * DT * P:(ib + 1) * DT * P])
                xib.append(xt)

            separts = persist.tile([P, NB * N_JC], F32, tag="separts")

            # jc-major: compute on chunk jc overlaps the DMA of chunk jc+1
            for jc in range(N_JC):
                xp_tp = []
                for tp in range(NTP):
                    base = (jc * NTP + tp) * 2 * JC_W
                    xp = persist.tile([P, 2 * JC_W], F8, tag=f"xpnT{jc}_{tp}")
                    nc.sync.dma_start(out=xp[:],
                                      in_=xpnT_d[:, base:base + 2 * JC_W])
                    xp_tp.append(xp)
                for ib in range(NB):
                    x3 = xib[ib][:].rearrange("p (t m) -> p t m", t=DT)
                    ps = psum.tile([P, JC_W], F32, tag="ps")
                    for tp in range(NTP):
                        lhs3 = x3[:, 2 * tp:2 * tp + 2, :]
                        rhs3 = xp_tp[tp][:].rearrange("p (o c) -> p o c", o=2)
                        for c in range(JC_W // MM_N):
                            nc.tensor.matmul(
                                ps[:, c * MM_N:(c + 1) * MM_N],
                                lhs3,
                                rhs3[:, :, c * MM_N:(c + 1) * MM_N],
                                start=(tp == 0), stop=(tp == NTP - 1),
                                perf_mode=mybir.MatmulPerfMode.DoubleRow)
                    esc = escp.tile([P, JC_W], BF16, tag="esc")
                    nc.scalar.activation(
                        esc[:], ps[:], mybir.ActivationFunctionType.Exp,
                        scale=rx_sb[:, ib:ib + 1],
                        accum_out=separts[:, ib * N_JC + jc:
                                          ib * N_JC + jc + 1])

            # one reduction + one Ln for all row blocks (single table load)
            se_all = persist.tile([P, NB], F32, tag="se_all")
            nc.vector.reduce_sum(
                se_all[:], separts[:].rearrange("p (i j) -> p i j", j=N_JC),
                axis=mybir.AxisListType.X)
            neg_sb = persist.tile([P, NB], F32, tag="neg_sb")
            nc.scalar.activation(neg_sb[:], se_all[:],
                                 mybir.ActivationFunctionType.Ln)
            nc.sync.dma_start(out=neg_d[:], in_=neg_sb[:])

    nc.compile()
    return nc


_NC1 = None
_NC2 = None


def _programs():
    global _NC1, _NC2
    if _NC1 is None:
        _NC1 = _build_dispatch1()
    if _NC2 is None:
        _NC2 = _build_dispatch2()
    return _NC1, _NC2


def kernel(x, y, W, b, _timing=None):
    assert x.shape == (N, D) and y.shape == (N, D)
    assert W.shape == (D, D) and b.shape == (D,)
    nc1, nc2 = _programs()
    core_ids = list(range(N_CORES))

    x = np.asarray(x, dtype=np.float32)
    y_bf = np.asarray(y, dtype=np.float32).astype(NP_BF16)
    x_bf = x.astype(NP_BF16)
    x_f8 = x.astype(NP_F8)

    # augmented W' = [W | b | zeros] transposed: [DTA*128, D]
    wTa = np.zeros((DTA * P, D), dtype=NP_BF16)
    wTa[:D] = np.asarray(W, dtype=np.float32).astype(NP_BF16).T
    wTa[D] = np.asarray(b, dtype=np.float32).astype(NP_BF16)
    wTa_sw = _swizzle_pm(wTa)

    in_maps1 = []
    for i in range(N_CORES):
        sl = slice(i * NS, (i + 1) * NS)
        yTa = np.zeros((DTA * P, NS), dtype=NP_BF16)
        yTa[:D] = y_bf[sl].T
        yTa[D] = NP_BF16(1.0)
        in_maps1.append({
            "yT": _swizzle_pm(yTa),
            "wT": wTa_sw,
            "xin": _swizzle_pm(x_bf[sl]),
        })
    r1 = run_bass_kernel_spmd(nc1, in_maps1, core_ids)
    if _timing is not None:
        _timing["d1"] = r1.exec_time_ns

    xpn = np.concatenate(
        [_unswizzle_pm(r1.results[i]["xpn"].astype(NP_BF16, copy=False), NB)
         for i in range(N_CORES)], axis=0)          # [N, D] bf16
    pos = np.concatenate(
        [r1.results[i]["stat"][:, :NB].T.ravel() for i in range(N_CORES)])

    # fp8 scores operand: 32 * xpn^T, swizzled to [p][jc][tp][o][c]
    xpn8T = np.ascontiguousarray(
        (xpn.astype(np.float32) * XPN_SCALE).astype(NP_F8).T)   # [D, N]
    xpnT_sw = np.ascontiguousarray(
        xpn8T.reshape(NTP, 2, P, N_JC, JC_W).transpose(2, 3, 0, 1, 4)
        .reshape(P, DT * N))

    in_maps2 = []
    for i in range(N_CORES):
        sl = slice(i * NS, (i + 1) * NS)
        rx_sw = np.ascontiguousarray(
            r1.results[i]["stat"][:, NB:] / np.float32(XPN_SCALE))
        # xT ib-major: [p, ib, t, m]
        xT8 = np.ascontiguousarray(x_f8[sl].T)            # [D, NS]
        xT_sw = np.ascontiguousarray(
            xT8.reshape(DT, P, NB, P).transpose(1, 2, 0, 3)
            .reshape(P, DT * NS))
        in_maps2.append({
            "xT": xT_sw,
            "xpnT": xpnT_sw,
            "rxv": rx_sw,
        })
    r2 = run_bass_kernel_spmd(nc2, in_maps2, core_ids)
    if _timing is not None:
        _timing["d2"] = r2.exec_time_ns

    neg = np.concatenate(
        [r2.results[i]["negv"].T.ravel() for i in range(N_CORES)])
    loss = np.mean(neg.astype(np.float64)) - np.mean(pos.astype(np.float64))
    return np.asarray(loss, dtype=np.float32)


# revision 20
# speedup vs baseline: 1.0438x; 1.0306x over previous
"""CPC InfoNCE loss kernel for 8x Trainium2 NeuronCores.

Math (reference):
    x_pred = y @ W.T + b                       [N, D]
    xpn    = x_pred / ||x_pred||_rows          [N, D]
    xn     = x / ||x||_rows                    [N, D]
    pos_i  = xn_i . xpn_i
    neg_i  = logsumexp_j(xn_i . xpn_j)
    loss   = -mean(pos - neg)

Strategy (data-parallel over N across 8 cores, two SPMD dispatches):

  Dispatch 1 (bf16): core i computes its row-shard of x_pred.  The bias is
    folded into the matmul by augmenting the contraction dim on the host:
    y' = [y | 1 | 0...], W' = [W | b | 0...]  (K: 1024 -> 1152), so the PSUM
    result needs no eviction pass — the ACT engine squares it directly for
    row norms, scales it to normalized bf16 output, and the DVE computes
    pos via an elementwise product + row reduction.  rx = 1/||x_row|| is
    also produced here.

  Host: gather the 8 normalized shards, transpose to [D, N], scale by 32
    and quantize to fp8e4m3 (cosine-similarity scores tolerate fp8; 32x
    keeps unit-norm entries in e4m3's normal range; the 1/32 is folded into
    the per-row exp scale).

  Dispatch 2 (fp8 + DoubleRow): core i computes its scores block
    u = x8_shard @ xpn8^T with DoubleRow matmuls (2 fp8 contraction rows
    per PE cell -> half the matmul instructions), then exp(u * rx_i/32)
    fused on the ACT engine (per-partition scale + row-accumulate), one Ln
    at the end -> neg rows.  exp without max-subtraction is safe: scores
    are cosine similarities in [-1, 1].

  Host: loss = mean(neg) - mean(pos).

All large tensors are pre-swizzled on the host into partition-major
[128, *] layouts so each tensor (or pipeline chunk) loads in one large
DMA (~2us fixed cost per DMA otherwise dominates), split across the two
HWDGE rings (sync + scalar) and the SWDGE ring (gpsimd).
"""

import sys

if "/opt/trn_rl_repo" not in sys.path:
    sys.path.insert(0, "/opt/trn_rl_repo")

import numpy as np
import ml_dtypes

import concourse.bass as bass
import concourse.bacc as bacc
import concourse.mybir as mybir
import concourse.tile as tile
from concourse.bass_utils import run_bass_kernel_spmd

BF16 = mybir.dt.bfloat16
F32 = mybir.dt.float32
F8 = mybir.dt.float8e4
NP_BF16 = ml_dtypes.bfloat16
NP_F8 = ml_dtypes.float8_e4m3fn

N_CORES = 8
N = 8192
D = 1024
NS = N // N_CORES  # rows per core = 1024
P = 128  # partitions
NB = NS // P  # row blocks per core = 8
DT = D // P  # contraction tiles = 8
DTA = DT + 1  # augmented contraction tiles (bias row + zero pad)
NTP = DT // 2  # DoubleRow tile pairs = 4
MM_N = 512  # moving free dim per matmul (one fp32 PSUM bank)
JC_W = 2048  # scores column chunk (4 PSUM banks, one ACT call)
N_JC = N // JC_W  # 4 chunks of the full N columns
XPN_SCALE = 32.0  # fp8 pre-scale for unit-norm rows


def _swizzle_pm(a):
    """[R*128, C] row-major -> [128, R*C] partition-major (tile r at columns
    r*C:(r+1)*C), so the whole tensor loads as one [128, R*C] DMA."""
    r8, c = a.shape[0] // P, a.shape[1]
    return np.ascontiguousarray(
        a.reshape(r8, P, c).transpose(1, 0, 2).reshape(P, r8 * c))


def _unswizzle_pm(a, r8):
    """Inverse of _swizzle_pm."""
    c = a.shape[1] // r8
    return np.ascontiguousarray(
        a.reshape(P, r8, c).transpose(1, 0, 2).reshape(r8 * P, c))


def _build_dispatch1():
    nc = bacc.Bacc("TRN2", target_bir_lowering=False, debug=False,
                   num_devices=N_CORES)
    yT_d = nc.dram_tensor("yT", [P, DTA * NS], BF16, kind="ExternalInput")
    wT_d = nc.dram_tensor("wT", [P, DTA * D], BF16, kind="ExternalInput")
    x_d = nc.dram_tensor("xin", [P, NB * D], BF16, kind="ExternalInput")
    xpn_d = nc.dram_tensor("xpn", [P, NB * D], BF16, kind="ExternalOutput")
    # stat: columns [0:NB] = pos, [NB:2NB] = rx
    stat_d = nc.dram_tensor("stat", [P, 2 * NB], F32, kind="ExternalOutput")

    with tile.TileContext(nc) as tc:
        with (
            tc.tile_pool(name="persist", bufs=1) as persist,
            tc.tile_pool(name="scratch", bufs=3) as scratch,
            tc.tile_pool(name="stats", bufs=NB) as stats,
            tc.tile_pool(name="psum", bufs=3,
                         space=bass.MemorySpace.PSUM) as psum,
        ):
            # split loads across rings; keep the ACT (scalar) queue free of
            # DMA triggers — it is d1's bottleneck engine
            yts, wts = [], []
            for t in range(DTA):
                yt = persist.tile([P, NS], BF16, tag=f"yT{t}")
                nc.sync.dma_start(out=yt[:], in_=yT_d[:, t * NS:(t + 1) * NS])
                yts.append(yt)
                wt = persist.tile([P, D], BF16, tag=f"wT{t}")
                nc.gpsimd.dma_start(out=wt[:], in_=wT_d[:, t * D:(t + 1) * D])
                wts.append(wt)
            x_sb = persist.tile([P, NB * D], BF16, tag="x")
            nc.gpsimd.dma_start(out=x_sb[:], in_=x_d[:])

            xpn_all = persist.tile([P, NB * D], BF16, tag="xpn_all")
            stat_all = persist.tile([P, 2 * NB], F32, tag="stat_all")

            for nb in range(NB):
                pp = psum.tile([P, D], F32, tag="pp")
                for t in range(DTA):
                    lhsT = yts[t][:, nb * P:(nb + 1) * P]
                    for c in range(D // MM_N):
                        nc.tensor.matmul(
                            pp[:, c * MM_N:(c + 1) * MM_N], lhsT,
                            wts[t][:, c * MM_N:(c + 1) * MM_N],
                            start=(t == 0), stop=(t == DTA - 1))

                # row sumsq -> 1/norm (ACT reads PSUM directly)
                sq = scratch.tile([P, D], F32, tag="sq")
                ss = stats.tile([P, 1], F32, tag="ss")
                nc.scalar.activation(sq[:], pp[:],
                                     mybir.ActivationFunctionType.Square,
                                     accum_out=ss[:])
                nrm = stats.tile([P, 1], F32, tag="nrm")
                nc.scalar.activation(nrm[:], ss[:],
                                     mybir.ActivationFunctionType.Sqrt)
                rpn = stats.tile([P, 1], F32, tag="rpn")
                nc.vector.reciprocal(rpn[:], nrm[:])

                # normalized rows -> bf16, streamed out per block
                nc.scalar.mul(xpn_all[:, nb * D:(nb + 1) * D], pp[:], rpn[:])
                nc.sync.dma_start(out=xpn_d[:, nb * D:(nb + 1) * D],
                                  in_=xpn_all[:, nb * D:(nb + 1) * D])

                # rx = 1/||x_row||
                xsq = scratch.tile([P, D], F32, tag="sq")
                ssx = stats.tile([P, 1], F32, tag="ssx")
                nc.scalar.activation(xsq[:], x_sb[:, nb * D:(nb + 1) * D],
                                     mybir.ActivationFunctionType.Square,
                                     accum_out=ssx[:])
                nx = stats.tile([P, 1], F32, tag="nx")
                nc.scalar.activation(nx[:], ssx[:],
                                     mybir.ActivationFunctionType.Sqrt)
                rx = stats.tile([P, 1], F32, tag="rx")
                nc.vector.reciprocal(rx[:], nx[:])
                nc.vector.tensor_copy(stat_all[:, NB + nb:NB + nb + 1], rx[:])

                # pos = (x_row . x_pred_row) * rpn * rx
                pd_scr = scratch.tile([P, D], F32, tag="sq")
                nc.vector.tensor_mul(pd_scr[:], x_sb[:, nb * D:(nb + 1) * D],
                                     pp[:])
                posdot = stats.tile([P, 1], F32, tag="posdot")
                nc.vector.reduce_sum(posdot[:], pd_scr[:],
                                     axis=mybir.AxisListType.X)
                t1 = stats.tile([P, 1], F32, tag="t1")
                nc.vector.tensor_mul(t1[:], posdot[:], rpn[:])
                nc.vector.tensor_mul(stat_all[:, nb:nb + 1], t1[:], rx[:])

            nc.gpsimd.dma_start(out=stat_d[:], in_=stat_all[:])

    nc.compile()
    return nc


def _build_dispatch2():
    nc = bacc.Bacc("TRN2", target_bir_lowering=False, debug=False,
                   num_devices=N_CORES)
    xT_d = nc.dram_tensor("xT", [P, DT * NS], F8, kind="ExternalInput")
    # layout: [p][jc][tp][o][c] blocks, each (jc, tp) block = [128, 2*JC_W]
    xpnT_d = nc.dram_tensor("xpnT", [P, DT * N], F8, kind="ExternalInput")
    rx_d = nc.dram_tensor("rxv", [P, NB], F32, kind="ExternalInput")
    neg_d = nc.dram_tensor("negv", [P, NB], F32, kind="ExternalOutput")

    with tile.TileContext(nc) as tc:
        with (
            tc.tile_pool(name="persist", bufs=1) as persist,
            tc.tile_pool(name="esc", bufs=2) as escp,
            tc.tile_pool(name="psum", bufs=2,
                         space=bass.MemorySpace.PSUM) as psum,
        ):
            rx_sb = persist.tile([P, NB], F32, tag="rx")
            nc.gpsimd.dma_start(out=rx_sb[:], in_=rx_d[:])
            # x^T loaded as per-ib chunks (ib-major host layout) so the first
            # row block's matmuls only wait on a 128 KB load
            xib = []
            for ib in range(NB):
                xt = persist.tile([P, DT * P], F8, tag=f"xib{ib}",
                                  name=f"xib{ib}")
                nc.gpsimd.dma_start(
                    out=xt[:], in_=xT_d[:, ib * DT * P:(ib + 1) * DT * P])
                xib.append(xt)

            separts = persist.tile([P, NB * N_JC], F32, tag="separts")

            # jc-major: compute on chunk jc overlaps the DMA of chunk jc+1
            for jc in range(N_JC):
                xp_tp = []
                for tp in range(NTP):
                    base = (jc * NTP + tp) * 2 * JC_W
                    xp = persist.tile([P, 2 * JC_W], F8, tag=f"xpnT{jc}_{tp}")
                    nc.sync.dma_start(out=xp[:],
                                      in_=xpnT_d[:, base:base + 2 * JC_W])
                    xp_tp.append(xp)
                for ib in range(NB):
                    x3 = xib[ib][:].rearrange("p (t m) -> p t m", t=DT)
                    ps = psum.tile([P, JC_W], F32, tag="ps")
                    for tp in range(NTP):
                        lhs3 = x3[:, 2 * tp:2 * tp + 2, :]
                        rhs3 = xp_tp[tp][:].rearrange("p (o c) -> p o c", o=2)
                        for c in range(JC_W // MM_N):
                            nc.tensor.matmul(
                                ps[:, c * MM_N:(c + 1) * MM_N],
                                lhs3,
                                rhs3[:, :, c * MM_N:(c + 1) * MM_N],
                                start=(tp == 0), stop=(tp == NTP - 1),
                                perf_mode=mybir.MatmulPerfMode.DoubleRow)
                    esc = escp.tile([P, JC_W], BF16, tag="esc")
                    nc.scalar.activation(
                        esc[:], ps[:], mybir.ActivationFunctionType.Exp,
                        scale=rx_sb[:, ib:ib + 1],
                        accum_out=separts[:, ib * N_JC + jc:
                                          ib * N_JC + jc + 1])

            # one reduction + one Ln for all row blocks (single table load)
            se_all = persist.tile([P, NB], F32, tag="se_all")
            nc.vector.reduce_sum(
                se_all[:], separts[:].rearrange("p (i j) -> p i j", j=N_JC),
                axis=mybir.AxisListType.X)
            neg_sb = persist.tile([P, NB], F32, tag="neg_sb")
            nc.scalar.activation(neg_sb[:], se_all[:],
                                 mybir.ActivationFunctionType.Ln)
            nc.sync.dma_start(out=neg_d[:], in_=neg_sb[:])

    nc.compile()
    return nc


_NC1 = None
_NC2 = None


def _programs():
    global _NC1, _NC2
    if _NC1 is None:
        _NC1 = _build_dispatch1()
    if _NC2 is None:
        _NC2 = _build_dispatch2()
    return _NC1, _NC2


def kernel(x, y, W, b, _timing=None):
    assert x.shape == (N, D) and y.shape == (N, D)
    assert W.shape == (D, D) and b.shape == (D,)
    nc1, nc2 = _programs()
    core_ids = list(range(N_CORES))

    x = np.asarray(x, dtype=np.float32)
    y_bf = np.asarray(y, dtype=np.float32).astype(NP_BF16)
    x_bf = x.astype(NP_BF16)
    x_f8 = x.astype(NP_F8)

    # augmented W' = [W | b | zeros] transposed: [DTA*128, D]
    wTa = np.zeros((DTA * P, D), dtype=NP_BF16)
    wTa[:D] = np.asarray(W, dtype=np.float32).astype(NP_BF16).T
    wTa[D] = np.asarray(b, dtype=np.float32).astype(NP_BF16)
    wTa_sw = _swizzle_pm(wTa)

    in_maps1 = []
    for i in range(N_CORES):
        sl = slice(i * NS, (i + 1) * NS)
        yTa = np.zeros((DTA * P, NS), dtype=NP_BF16)
        yTa[:D] = y_bf[sl].T
        yTa[D] = NP_BF16(1.0)
        in_maps1.append({
            "yT": _swizzle_pm(yTa),
            "wT": wTa_sw,
            "xin": _swizzle_pm(x_bf[sl]),
        })
    r1 = run_bass_kernel_spmd(nc1, in_maps1, core_ids)
    if _timing is not None:
        _timing["d1"] = r1.exec_time_ns

    xpn = np.concatenate(
        [_unswizzle_pm(r1.results[i]["xpn"].astype(NP_BF16, copy=False), NB)
         for i in range(N_CORES)], axis=0)          # [N, D] bf16
    pos = np.concatenate(
        [r1.results[i]["stat"][:, :NB].T.ravel() for i in range(N_CORES)])

    # fp8 scores operand: 32 * xpn^T, swizzled to [p][jc][tp][o][c]
    xpn8T = np.ascontiguousarray(
        (xpn.astype(np.float32) * XPN_SCALE).astype(NP_F8).T)   # [D, N]
    xpnT_sw = np.ascontiguousarray(
        xpn8T.reshape(NTP, 2, P, N_JC, JC_W).transpose(2, 3, 0, 1, 4)
        .reshape(P, DT * N))

    in_maps2 = []
    for i in range(N_CORES):
        sl = slice(i * NS, (i + 1) * NS)
        rx_sw = np.ascontiguousarray(
            r1.results[i]["stat"][:, NB:] / np.float32(XPN_SCALE))
        # xT ib-major: [p, ib, t, m]
        xT8 = np.ascontiguousarray(x_f8[sl].T)            # [D, NS]
        xT_sw = np.ascontiguousarray(
            xT8.reshape(DT, P, NB, P).transpose(1, 2, 0, 3)
            .reshape(P, DT * NS))
        in_maps2.append({
            "xT": xT_sw,
            "xpnT": xpnT_sw,
            "rxv": rx_sw,
        })
    r2 = run_bass_kernel_spmd(nc2, in_maps2, core_ids)
    if _timing is not None:
        _timing["d2"] = r2.exec_time_ns

    neg = np.concatenate(
        [r2.results[i]["negv"].T.ravel() for i in range(N_CORES)])
    loss = np.mean(neg.astype(np.float64)) - np.mean(pos.astype(np.float64))
    return np.asarray(loss, dtype=np.float32)


# revision 22
# speedup vs baseline: 1.0535x; 1.0093x over previous
"""CPC InfoNCE loss kernel for 8x Trainium2 NeuronCores.

Math (reference):
    x_pred = y @ W.T + b                       [N, D]
    xpn    = x_pred / ||x_pred||_rows          [N, D]
    xn     = x / ||x||_rows                    [N, D]
    pos_i  = xn_i . xpn_i
    neg_i  = logsumexp_j(xn_i . xpn_j)
    loss   = -mean(pos - neg)

Strategy (data-parallel over N across 8 cores, two SPMD dispatches):

  Dispatch 1 (bf16): core i computes its row-shard of x_pred.  The bias is
    folded into the matmul by augmenting the contraction dim on the host:
    y' = [y | 1 | 0...], W' = [W | b | 0...]  (K: 1024 -> 1152), so the PSUM
    result needs no eviction pass — the ACT engine squares it directly for
    row norms, scales it to normalized bf16 output, and the DVE computes
    pos via an elementwise product + row reduction.  rx = 1/||x_row|| is
    also produced here.

  Host: gather the 8 normalized shards, transpose to [D, N], scale by 32
    and quantize to fp8e4m3 (cosine-similarity scores tolerate fp8; 32x
    keeps unit-norm entries in e4m3's normal range; the 1/32 is folded into
    the per-row exp scale).

  Dispatch 2 (fp8 + DoubleRow): core i computes its scores block
    u = x8_shard @ xpn8^T with DoubleRow matmuls (2 fp8 contraction rows
    per PE cell -> half the matmul instructions), then exp(u * rx_i/32)
    fused on the ACT engine (per-partition scale + row-accumulate), one Ln
    at the end -> neg rows.  exp without max-subtraction is safe: scores
    are cosine similarities in [-1, 1].

  Host: loss = mean(neg) - mean(pos).

All large tensors are pre-swizzled on the host into partition-major
[128, *] layouts so each tensor (or pipeline chunk) loads in one large
DMA (~2us fixed cost per DMA otherwise dominates), split across the sync
HWDGE ring and the gpsimd SWDGE ring.  DMA triggers occupy the issuing
engine's queue for the whole transfer, so the ACT (scalar) queue — the
bottleneck engine in dispatch 1 and the exp engine in dispatch 2 — issues
no DMAs at all.
"""

import sys

if "/opt/trn_rl_repo" not in sys.path:
    sys.path.insert(0, "/opt/trn_rl_repo")

import numpy as np
import ml_dtypes

import concourse.bass as bass
import concourse.bacc as bacc
import concourse.mybir as mybir
import concourse.tile as tile
from concourse.bass_utils import run_bass_kernel_spmd

BF16 = mybir.dt.bfloat16
F32 = mybir.dt.float32
F8 = mybir.dt.float8e4
NP_BF16 = ml_dtypes.bfloat16
NP_F8 = ml_dtypes.float8_e4m3fn

N_CORES = 8
N = 8192
D = 1024
NS = N // N_CORES  # rows per core = 1024
P = 128  # partitions
NB = NS // P  # row blocks per core = 8
DT = D // P  # contraction tiles = 8
DTA = DT + 1  # augmented contraction tiles (bias row + zero pad)
NTP = DT // 2  # DoubleRow tile pairs = 4
MM_N = 512  # moving free dim per matmul (one fp32 PSUM bank)
JC_W = 2048  # scores column chunk (4 PSUM banks, one ACT call)
N_JC = N // JC_W  # 4 chunks of the full N columns
XPN_SCALE = 32.0  # fp8 pre-scale for unit-norm rows


def _swizzle_pm(a):
    """[R*128, C] row-major -> [128, R*C] partition-major (tile r at columns
    r*C:(r+1)*C), so the whole tensor loads as one [128, R*C] DMA."""
    r8, c = a.shape[0] // P, a.shape[1]
    return np.ascontiguousarray(
        a.reshape(r8, P, c).transpose(1, 0, 2).reshape(P, r8 * c))


def _unswizzle_pm(a, r8):
    """Inverse of _swizzle_pm."""
    c = a.shape[1] // r8
    return np.ascontiguousarray(
        a.reshape(P, r8, c).transpose(1, 0, 2).reshape(r8 * P, c))


def _build_dispatch1():
    nc = bacc.Bacc("TRN2", target_bir_lowering=False, debug=False,
                   num_devices=N_CORES)
    yT_d = nc.dram_tensor("yT", [P, DTA * NS], BF16, kind="ExternalInput")
    wT_d = nc.dram_tensor("wT", [P, DTA * D], BF16, kind="ExternalInput")
    x_d = nc.dram_tensor("xin", [P, NB * D], BF16, kind="ExternalInput")
    xpn_d = nc.dram_tensor("xpn", [P, NB * D], BF16, kind="ExternalOutput")
    # stat: columns [0:NB] = pos, [NB:2NB] = rx
    stat_d = nc.dram_tensor("stat", [P, 2 * NB], F32, kind="ExternalOutput")

    with tile.TileContext(nc) as tc:
        with (
            tc.tile_pool(name="persist", bufs=1) as persist,
            tc.tile_pool(name="scratch", bufs=3) as scratch,
            tc.tile_pool(name="stats", bufs=NB) as stats,
            tc.tile_pool(name="psum", bufs=3,
                         space=bass.MemorySpace.PSUM) as psum,
        ):
            # split loads across rings; keep the ACT (scalar) queue free of
            # DMA triggers — it is d1's bottleneck engine
            yts, wts = [], []
            for t in range(DTA):
                yt = persist.tile([P, NS], BF16, tag=f"yT{t}")
                nc.sync.dma_start(out=yt[:], in_=yT_d[:, t * NS:(t + 1) * NS])
                yts.append(yt)
                wt = persist.tile([P, D], BF16, tag=f"wT{t}")
                nc.gpsimd.dma_start(out=wt[:], in_=wT_d[:, t * D:(t + 1) * D])
                wts.append(wt)
            # x loaded per-nb so the first row block's rx/pos chain starts
            # as soon as its 0.25 MB chunk lands
            x_sb = persist.tile([P, NB * D], BF16, tag="x")
            for nb in range(NB):
                nc.gpsimd.dma_start(out=x_sb[:, nb * D:(nb + 1) * D],
                                    in_=x_d[:, nb * D:(nb + 1) * D])

            xpn_all = persist.tile([P, NB * D], BF16, tag="xpn_all")
            stat_all = persist.tile([P, 2 * NB], F32, tag="stat_all")

            for nb in range(NB):
                pp = psum.tile([P, D], F32, tag="pp")
                for t in range(DTA):
                    lhsT = yts[t][:, nb * P:(nb + 1) * P]
                    for c in range(D // MM_N):
                        nc.tensor.matmul(
                            pp[:, c * MM_N:(c + 1) * MM_N], lhsT,
                            wts[t][:, c * MM_N:(c + 1) * MM_N],
                            start=(t == 0), stop=(t == DTA - 1))

                # row sumsq -> 1/norm (ACT reads PSUM directly)
                sq = scratch.tile([P, D], F32, tag="sq")
                ss = stats.tile([P, 1], F32, tag="ss")
                nc.scalar.activation(sq[:], pp[:],
                                     mybir.ActivationFunctionType.Square,
                                     accum_out=ss[:])
                nrm = stats.tile([P, 1], F32, tag="nrm")
                nc.scalar.activation(nrm[:], ss[:],
                                     mybir.ActivationFunctionType.Sqrt)
                rpn = stats.tile([P, 1], F32, tag="rpn")
                nc.vector.reciprocal(rpn[:], nrm[:])

                # normalized rows -> bf16, streamed out per block
                nc.scalar.mul(xpn_all[:, nb * D:(nb + 1) * D], pp[:], rpn[:])
                nc.sync.dma_start(out=xpn_d[:, nb * D:(nb + 1) * D],
                                  in_=xpn_all[:, nb * D:(nb + 1) * D])

                # rx = 1/||x_row||
                xsq = scratch.tile([P, D], F32, tag="sq")
                ssx = stats.tile([P, 1], F32, tag="ssx")
                nc.scalar.activation(xsq[:], x_sb[:, nb * D:(nb + 1) * D],
                                     mybir.ActivationFunctionType.Square,
                                     accum_out=ssx[:])
                nx = stats.tile([P, 1], F32, tag="nx")
                nc.scalar.activation(nx[:], ssx[:],
                                     mybir.ActivationFunctionType.Sqrt)
                rx = stats.tile([P, 1], F32, tag="rx")
                nc.vector.reciprocal(rx[:], nx[:])
                nc.vector.tensor_copy(stat_all[:, NB + nb:NB + nb + 1], rx[:])

                # pos = (x_row . x_pred_row) * rpn * rx
                pd_scr = scratch.tile([P, D], F32, tag="sq")
                nc.vector.tensor_mul(pd_scr[:], x_sb[:, nb * D:(nb + 1) * D],
                                     pp[:])
                posdot = stats.tile([P, 1], F32, tag="posdot")
                nc.vector.reduce_sum(posdot[:], pd_scr[:],
                                     axis=mybir.AxisListType.X)
                t1 = stats.tile([P, 1], F32, tag="t1")
                nc.vector.tensor_mul(t1[:], posdot[:], rpn[:])
                nc.vector.tensor_mul(stat_all[:, nb:nb + 1], t1[:], rx[:])

            nc.gpsimd.dma_start(out=stat_d[:], in_=stat_all[:])

    nc.compile()
    return nc


def _build_dispatch2():
    nc = bacc.Bacc("TRN2", target_bir_lowering=False, debug=False,
                   num_devices=N_CORES)
    xT_d = nc.dram_tensor("xT", [P, DT * NS], F8, kind="ExternalInput")
    # layout: [p][jc][tp][o][c] blocks, each (jc, tp) block = [128, 2*JC_W]
    xpnT_d = nc.dram_tensor("xpnT", [P, DT * N], F8, kind="ExternalInput")
    rx_d = nc.dram_tensor("rxv", [P, NB], F32, kind="ExternalInput")
    neg_d = nc.dram_tensor("negv", [P, NB], F32, kind="ExternalOutput")

    with tile.TileContext(nc) as tc:
        with (
            tc.tile_pool(name="persist", bufs=1) as persist,
            tc.tile_pool(name="esc", bufs=2) as escp,
            tc.tile_pool(name="psum", bufs=2,
                         space=bass.MemorySpace.PSUM) as psum,
        ):
            rx_sb = persist.tile([P, NB], F32, tag="rx")
            nc.gpsimd.dma_start(out=rx_sb[:], in_=rx_d[:])
            # x^T loaded as per-ib chunks (ib-major host layout) so the first
            # row block's matmuls only wait on a 128 KB load
            xib = []
            for ib in range(NB):
                xt = persist.tile([P, DT * P], F8, tag=f"xib{ib}",
                                  name=f"xib{ib}")
                nc.gpsimd.dma_start(
                    out=xt[:], in_=xT_d[:, ib * DT * P:(ib + 1) * DT * P])
                xib.append(xt)

            separts = persist.tile([P, NB * N_JC], F32, tag="separts")

            # jc-major: compute on chunk jc overlaps the DMA of chunk jc+1
            for jc in range(N_JC):
                xp_tp = []
                for tp in range(NTP):
                    base = (jc * NTP + tp) * 2 * JC_W
                    xp = persist.tile([P, 2 * JC_W], F8, tag=f"xpnT{jc}_{tp}")
                    nc.sync.dma_start(out=xp[:],
                                      in_=xpnT_d[:, base:base + 2 * JC_W])
                    xp_tp.append(xp)
                for ib in range(NB):
                    x3 = xib[ib][:].rearrange("p (t m) -> p t m", t=DT)
                    ps = psum.tile([P, JC_W], F32, tag="ps")
                    for tp in range(NTP):
                        lhs3 = x3[:, 2 * tp:2 * tp + 2, :]
                        rhs3 = xp_tp[tp][:].rearrange("p (o c) -> p o c", o=2)
                        for c in range(JC_W // MM_N):
                            nc.tensor.matmul(
                                ps[:, c * MM_N:(c + 1) * MM_N],
                                lhs3,
                                rhs3[:, :, c * MM_N:(c + 1) * MM_N],
                                start=(tp == 0), stop=(tp == NTP - 1),
                                perf_mode=mybir.MatmulPerfMode.DoubleRow)
                    esc = escp.tile([P, JC_W], BF16, tag="esc")
                    nc.scalar.activation(
                        esc[:], ps[:], mybir.ActivationFunctionType.Exp,
                        scale=rx_sb[:, ib:ib + 1],
                        accum_out=separts[:, ib * N_JC + jc:
                                          ib * N_JC + jc + 1])

            # one reduction + one Ln for all row blocks (single table load)
            se_all = persist.tile([P, NB], F32, tag="se_all")
            nc.vector.reduce_sum(
                se_all[:], separts[:].rearrange("p (i j) -> p i j", j=N_JC),
                axis=mybir.AxisListType.X)
            neg_sb = persist.tile([P, NB], F32, tag="neg_sb")
            nc.scalar.activation(neg_sb[:], se_all[:],
                                 mybir.ActivationFunctionType.Ln)
            nc.sync.dma_start(out=neg_d[:], in_=neg_sb[:])

    nc.compile()
    return nc


_NC1 = None
_NC2 = None


def _programs():
    global _NC1, _NC2
    if _NC1 is None:
        _NC1 = _build_dispatch1()
    if _NC2 is None:
        _NC2 = _build_dispatch2()
    return _NC1, _NC2


def kernel(x, y, W, b, _timing=None):
    assert x.shape == (N, D) and y.shape == (N, D)
    assert W.shape == (D, D) and b.shape == (D,)
    nc1, nc2 = _programs()
    core_ids = list(range(N_CORES))

    x = np.asarray(x, dtype=np.float32)
    y_bf = np.asarray(y, dtype=np.float32).astype(NP_BF16)
    x_bf = x.astype(NP_BF16)
    x_f8 = x.astype(NP_F8)

    # augmented W' = [W | b | zeros] transposed: [DTA*128, D]
    wTa = np.zeros((DTA * P, D), dtype=NP_BF16)
    wTa[:D] = np.asarray(W, dtype=np.float32).astype(NP_BF16).T
    wTa[D] = np.asarray(b, dtype=np.float32).astype(NP_BF16)
    wTa_sw = _swizzle_pm(wTa)

    in_maps1 = []
    for i in range(N_CORES):
        sl = slice(i * NS, (i + 1) * NS)
        yTa = np.zeros((DTA * P, NS), dtype=NP_BF16)
        yTa[:D] = y_bf[sl].T
        yTa[D] = NP_BF16(1.0)
        in_maps1.append({
            "yT": _swizzle_pm(yTa),
            "wT": wTa_sw,
            "xin": _swizzle_pm(x_bf[sl]),
        })
    r1 = run_bass_kernel_spmd(nc1, in_maps1, core_ids)
    if _timing is not None:
        _timing["d1"] = r1.exec_time_ns

    xpn = np.concatenate(
        [_unswizzle_pm(r1.results[i]["xpn"].astype(NP_BF16, copy=False), NB)
         for i in range(N_CORES)], axis=0)          # [N, D] bf16
    pos = np.concatenate(
        [r1.results[i]["stat"][:, :NB].T.ravel() for i in range(N_CORES)])

    # fp8 scores operand: 32 * xpn^T, swizzled to [p][jc][tp][o][c]
    xpn8T = np.ascontiguousarray(
        (xpn.astype(np.float32) * XPN_SCALE).astype(NP_F8).T)   # [D, N]
    xpnT_sw = np.ascontiguousarray(
        xpn8T.reshape(NTP, 2, P, N_JC, JC_W).transpose(2, 3, 0, 1, 4)
        .reshape(P, DT * N))

    in_maps2 = []
    for i in range(N_CORES):
        sl = slice(i * NS, (i + 1) * NS)
        rx_sw = np.ascontiguousarray(
            r1.results[i]["stat"][:, NB:] / np.float32(XPN_SCALE))
        # xT ib-major: [p, ib, t, m]
        xT8 = np.ascontiguousarray(x_f8[sl].T)            # [D, NS]
        xT_sw = np.ascontiguousarray(
            xT8.reshape(DT, P, NB, P).transpose(1, 2, 0, 3)
            .reshape(P, DT * NS))
        in_maps2.append({
            "xT": xT_sw,
            "xpnT": xpnT_sw,
            "rxv": rx_sw,
        })
    r2 = run_bass_kernel_spmd(nc2, in_maps2, core_ids)
    if _timing is not None:
        _timing["d2"] = r2.exec_time_ns

    neg = np.concatenate(
        [r2.results[i]["negv"].T.ravel() for i in range(N_CORES)])
    loss = np.mean(neg.astype(np.float64)) - np.mean(pos.astype(np.float64))
    return np.asarray(loss, dtype=np.float32)
